# revision 10
# baseline (speedup 1.0000x reference)
"""Causal self-attention (B=4, T=2048, C=1024, H=16) on 8 Trainium2 NeuronCores.

Sharding: data-parallel over batch (4) x tensor-parallel over head halves (2)
= 8 cores. Core c handles batch b = c//2 and heads [8*(c%2), 8*(c%2)+8).

Structure: the attention@V product uses PROBS-STATIONARY matmuls:
  av[q, d+1] += pr_chunk[key, q].T @ v_aug[key, d+1]
so each matmul streams only 65 columns (v_aug incl. the ones column that
accumulates the softmax denominator) instead of 512 probability columns.
The av psum holds 4 interleaved accumulation groups per bank, so only the
bank's very first matmul carries start=True (start clears has_written for
the whole bank; per-element overwrite-where-unset handles the rest). The
attention output lands [q, d]; softmax normalize is a per-partition
reciprocal + broadcast multiply on DVE, and an xbar DMA-transpose per
head-pair restores the [d, q] layout the out-projection needs as its
stationary operand (the final pair uses PE transposes instead — the DMA
latency would sit on the end-of-kernel critical chain).

Dtypes: projection inputs, k^T/q^T/v/probs/attn/w_out bf16; psum fp32.

Scheduling: Act's exp stream paces attention; projection / out-projection
work is emitted through a filler queue pumped between the per-key-block
matmuls, sized by an emission-time Act/PE debt model with a pair-start
boost that covers the sc-ring wait on the previous pair's last exp. All
projection dep units are queued before the out-projection rows so the rows
survive as fill for qt3, where the Act-over-PE gap is largest. The av
batch for key block kb is emitted after sc(kb+AV_LAG) so the previous
pair's normalize has runway before the av psum ring is reused. DMA issue
is split across gpsimd/SWDGE (x^T) and sync/HWDGE (weights) with the
first (wv, xT) chunks interleaved at fine granularity, since transfers
serialize on the DMA engines; warm-up fp32 matmuls during the initial DMA
window start the PE p-state ramp clock.
"""
import sys

if "/opt/trn_rl_repo" not in sys.path:
    sys.path.insert(0, "/opt/trn_rl_repo")

import numpy as np

T = 2048
C = 1024
HLOC = 8          # heads per core
DK = 64
HD = HLOC * DK    # 512 local head dims
KC = C // 128     # 8 contraction chunks for the qkv projection
NMT = HD // 128   # 4 tiles of q^T / k^T rows
NVT = T // 128    # 16 v tiles
NQT = T // 512    # 4 q tiles of 512
SCALE = DK ** -0.5

PROJ_BF16 = True  # bf16 inputs for the qkv projection (x^T, w_q/k/v)

_CACHE = {}


def _build_nc(probs_bufs=5, proj_bf16=PROJ_BF16, pool_alloc_mode="stack",
              drbs_bufs=2, fill_bufs=2, qtp_bufs=2, attn_bufs=4, yp_bufs=8,
              DEBT_CLAMP=2000.0, QT_FLOOR=0, DEBT_FLOOR=150.0,
              START_BOOST=600.0, BOOST_QT=0, ACT_OVH=185.0, WARM_N=8,
              AV_LAG=4, RESERVE_MARGIN=None, BOOST_KB=3, END_BOOST=0.0,
              TP_KB=2):
    import concourse.mybir as mybir
    import concourse.tile as tile
    from concourse import bacc
    from concourse.masks import make_identity

    F32 = mybir.dt.float32
    F32R = mybir.dt.float32r
    BF16 = mybir.dt.bfloat16
    AF = mybir.ActivationFunctionType
    in_dt = BF16 if proj_bf16 else F32R

    nc = bacc.Bacc("TRN2", target_bir_lowering=False, debug=False, num_devices=8)
    xT = nc.dram_tensor("xT", [C, T], in_dt, kind="ExternalInput")
    # wk/wq are pre-transposed on the host to [partition, mt, kc, n] so the
    # mt=0 slices (all pair-0 needs) can be DMA'd first as one contiguous
    # transfer each
    wq = nc.dram_tensor("wq", [128, NMT, KC, 128], in_dt, kind="ExternalInput")
    wk = nc.dram_tensor("wk", [128, NMT, KC, 128], in_dt, kind="ExternalInput")
    wv = nc.dram_tensor("wv", [C, HD], in_dt, kind="ExternalInput")
    wo = nc.dram_tensor("wo", [HD, C], BF16, kind="ExternalInput")
    y = nc.dram_tensor("y", [T, C], BF16, kind="ExternalOutput")

    with tile.TileContext(nc, pool_alloc_mode=pool_alloc_mode) as tc:
        with tc.tile_pool(name="const", bufs=1) as const, \
             tc.tile_pool(name="qkv", bufs=1) as qkv, \
             tc.tile_pool(name="qTp", bufs=qtp_bufs) as qTp, \
             tc.tile_pool(name="xtw", bufs=1) as xtw, \
             tc.tile_pool(name="wpool", bufs=1) as wpool, \
             tc.tile_pool(name="attnp", bufs=attn_bufs) as attnp, \
             tc.tile_pool(name="asbp", bufs=2) as asbp, \
             tc.tile_pool(name="probsp", bufs=probs_bufs) as probsp, \
             tc.tile_pool(name="drp", bufs=drbs_bufs) as drp, \
             tc.tile_pool(name="wop", bufs=1) as wop, \
             tc.tile_pool(name="yp", bufs=yp_bufs) as yp, \
             tc.tile_pool(name="psp", bufs=2, space="PSUM") as psp:
            # ---- constants (instructions for identity/ones emitted after
            # the DMAs so nothing queues ahead of the Pool-issued x^T loads)
            cpack = const.tile([128, 65], F32)
            onecol_f = cpack[:, 0:1]
            ident = const.tile([128, 128], BF16)
            if WARM_N:
                # dummy fp32 matmuls (4 cycles/row) during the initial DMA
                # wait keep the PE busy so the p-state ramp completes before
                # real work arrives; their memset goes first so nothing
                # queues ahead of them
                cz = cpack[:, 1:65]
                nc.vector.memset(cz, 0.0)
                warm_ps = psp.tile([128, 512], F32, tag="fill", bufs=fill_bufs,
                                   name="warm")
                for _ in range(WARM_N):
                    nc.tensor.matmul(warm_ps[0:64, 0:64], cz, cz,
                                     start=True, stop=True)

            # ---- long-lived tiles ----
            kT_sb = qkv.tile([128, NMT, T], BF16)           # k^T: [head_dim, t]
            v_sb = qkv.tile([128, NVT, HLOC * 65], BF16)    # v_aug: ones col per head
            xT_sb = xtw.tile([128, KC, T], in_dt)
            wo_sb = wop.tile([128, NMT, C], BF16)

            wv_sb = wpool.tile([128, KC, HD], in_dt, tag="w")
            wk_sb = wpool.tile([128, NMT, KC, 128], in_dt, tag="w2")
            wq_sb = wpool.tile([128, NMT, KC, 128], in_dt, tag="w3")
            wv_re = wv.rearrange("(kc p) n -> p kc n", p=128)
            # DMA issue is split across two independent pipelines: x^T goes
            # through gpsimd (SWDGE — no shared-HWDGE serialization) while
            # the weights stream through the sync/HWDGE path in parallel.
            # Order within each path is by first use: the braid consumes
            # (wv[kc], xT[kc]) pairs in kc order.
            xT_re = xT.rearrange("(kc p) n -> p kc n", p=128)
            # transfers serialize on the DMA engines, so the (wv, xT) chunks
            # the braid consumes are interleaved at fine granularity up
            # front; chunk k's pair lands ~0.73us apart, just ahead of the
            # braid's ~0.85us/kc consumption
            for lo, hi in ((0, 1), (1, 2), (2, 4), (4, 6), (6, 8)):
                nc.sync.dma_start(out=wv_sb[:, lo:hi, :], in_=wv_re[:, lo:hi, :])
                nc.gpsimd.dma_start(out=xT_sb[:, lo:hi, 0:512],
                                    in_=xT_re[:, lo:hi, 0:512])
            for mt in range(NMT):
                nc.sync.dma_start(out=wk_sb[:, mt, :, :],
                                  in_=wk.ap()[:, mt, :, :])
                nc.sync.dma_start(out=wq_sb[:, mt, :, :],
                                  in_=wq.ap()[:, mt, :, :])
            nc.gpsimd.dma_start(out=xT_sb[:, :, 512:1024],
                                in_=xT_re[:, :, 512:1024])
            nc.gpsimd.dma_start(out=xT_sb[:, :, 1024:2048],
                                in_=xT_re[:, :, 1024:2048])
            wo_re = wo.rearrange("(kc p) n -> p kc n", p=128)
            nc.sync.dma_start(out=wo_sb, in_=wo_re)

            nc.vector.memset(onecol_f, 1.0)
            make_identity(nc, ident[:, :])
            # lower-triangular keep-mask (1.0 where col >= row): the causal
            # zeroing of the diagonal prob chunk runs as a 2x-mode DVE
            # multiply instead of a gpsimd affine_select, so end-of-pair av
            # batches don't queue behind a burst of Pool ops
            ltri = const.tile([128, 128], BF16)
            nc.gpsimd.memset(ltri, 1.0)
            nc.gpsimd.affine_select(
                out=ltri, in_=ltri,
                compare_op=mybir.AluOpType.is_ge,
                fill=0.0, base=0,
                pattern=[[1, 128]],
                channel_multiplier=-1)

            def fill_psum(name, shape=None, dtype=F32):
                return psp.tile(shape or [128, 512], dtype, tag="fill",
                                bufs=fill_bufs, name=name)

            # ---- filler units: projection / out-projection / transpose work
            # emitted as generators that yield (cost_cycles) after each PE
            # matmul, so attention can pump exactly enough PE work to cover
            # the Act-bound exp stream
            import collections as _co

            filler = _co.deque()   # (name, genfn, ready_fn)
            active = [None]
            done_units = set()
            debt = [0.0]
            PE_CYC = 1.0 / 2.4
            inv_ns = [0.0]      # filler inventory left (PE-ns)
            gap_rem = [0.0]     # Act-over-PE gap still ahead of us (ns)

            def _advance(force=False):
                while True:
                    if active[0] is None:
                        if not filler:
                            return False
                        nm, gf, ready = filler[0]
                        if ready is not None and not ready():
                            if force:
                                raise RuntimeError(f"unit {nm} forced before ready")
                            return False
                        active[0] = (nm, gf())
                        filler.popleft()
                    nm, g = active[0]
                    try:
                        cost = next(g)
                        spent = (cost or 512) * PE_CYC
                        debt[0] -= spent
                        inv_ns[0] -= spent
                        return True
                    except StopIteration:
                        done_units.update(nm.split("|"))
                        active[0] = None

            def drain(*names):
                while True:
                    missing = [nm for nm in names if nm not in done_units]
                    if not missing:
                        return
                    if not _advance(force=True) and missing:
                        missing = [nm for nm in names if nm not in done_units]
                        if missing:
                            raise RuntimeError(f"filler exhausted: {missing}")

            def _v_copy(i, ps):
                vt = v_sb[:, i, :].rearrange("p (h e) -> p h e", e=65)
                nc.vector.tensor_copy(
                    vt[:, :, 0:64], ps.rearrange("p (h d) -> p h d", d=64))
                nc.vector.tensor_copy(
                    vt[:, :, 64:65], onecol_f.broadcast_to([128, HLOC, 1]))

            def U_v(i):
                def g():
                    ps = fill_psum(f"psv{i}")
                    for kc in range(KC):
                        nc.tensor.matmul(
                            ps, xT_sb[:, kc, i * 128:(i + 1) * 128],
                            wv_sb[:, kc, :],
                            start=(kc == 0), stop=(kc == KC - 1))
                        yield 512
                    _v_copy(i, ps)
                return g

            def U_v_braid(i0):
                # v tiles i0..i0+3 interleaved at kc granularity so each
                # arriving xT column chunk unlocks 4 matmuls (prologue only:
                # borrows the idle "av" psum tag for 2 of its 4 live psums).
                def g():
                    pss = [fill_psum(f"psv{i0}"), fill_psum(f"psv{i0 + 1}"),
                           psp.tile([128, 512], F32, tag="av", bufs=2,
                                    name=f"psv{i0 + 2}"),
                           psp.tile([128, 512], F32, tag="av", bufs=2,
                                    name=f"psv{i0 + 3}")]
                    for kc in range(KC):
                        for j in range(4):
                            i = i0 + j
                            nc.tensor.matmul(
                                pss[j], xT_sb[:, kc, i * 128:(i + 1) * 128],
                                wv_sb[:, kc, :],
                                start=(kc == 0), stop=(kc == KC - 1))
                            yield 512
                    for j in range(4):
                        _v_copy(i0 + j, pss[j])
                return g

            def U_k(mt, c):
                # k^T rows [mt*128, +128), key columns [c*512, +512)
                def g():
                    ps = fill_psum(f"psk{mt}_{c}")
                    for kc in range(KC):
                        nc.tensor.matmul(
                            ps, wk_sb[:, mt, kc, :],
                            xT_sb[:, kc, c * 512:(c + 1) * 512],
                            start=(kc == 0), stop=(kc == KC - 1))
                        yield 512
                    nc.vector.tensor_copy(
                        kT_sb[:, mt, c * 512:(c + 1) * 512], ps)
                return g

            def U_q(mt, qt, qT_t):
                # q^T rows [mt*128, +128) for q block qt
                def g():
                    ps = fill_psum(f"psq{mt}_{qt}")
                    for kc in range(KC):
                        nc.tensor.matmul(
                            ps, wq_sb[:, mt, kc, :],
                            xT_sb[:, kc, qt * 512:(qt + 1) * 512],
                            start=(kc == 0), stop=(kc == KC - 1))
                        yield 512
                    nc.vector.tensor_copy(
                        qT_t[:, mt, (qt % 2) * 512:(qt % 2 + 1) * 512], ps)
                return g

            # pump pacing: Act ns per free element (steady-state clocks);
            # ACT_OVH is the per-instruction access overhead of an exp
            ACT_EL = 1.0 / 1.2

            qT_tiles = [qTp.tile([128, NMT, 1024], BF16, tag="qT",
                                 name=f"qT{n}") for n in range(2)]
            # normalized attention, transposed back to [d, q] (bf16)
            attn_tiles = [attnp.tile([128, NMT, 512], BF16, tag="attn",
                                     name=f"attn{qt}") for qt in range(NQT)]
            tp_done = [0] * NQT         # per qt: # pairs transposed+copied
            pending_tp = [None]         # previous pair's transpose closure

            def emit_attention_pair(qt, mt, qT_t):
                # head pair (2mt, 2mt+1) for q columns [qt*512, (qt+1)*512)
                nkb = qt * 4 + 4
                # probs-stationary attention@V accumulators: [q, 4sub, 64+1]
                av = [psp.tile([128, NMT, 65], F32, tag="av", bufs=2,
                               name=f"av{qt}_{mt}_{s}") for s in range(2)]
                prs = {}

                def emit_av(kb):
                    # attention@V for key block kb: pr(kb) is stationary,
                    # v_aug chunks stream 65 columns each.
                    # start=True clears has_written for the WHOLE bank, so
                    # only the bank's very first matmul may use it; the other
                    # j-groups' first writes overwrite-where-unset, which the
                    # per-element has_written bits handle correctly.
                    pr = prs.pop(kb)
                    kbl = kb - qt * 4
                    jmin = max(kbl, 0)
                    for s in range(2):
                        h = 2 * mt + s
                        for j in range(jmin, NMT):
                            nc.tensor.matmul(
                                av[s][:, j, :],
                                pr[:, s, j * 128:(j + 1) * 128],
                                v_sb[:, kb, h * 65:(h + 1) * 65],
                                start=(kb == 0 and j == 0),
                                stop=(kb == qt * 4 + j),
                                skip_group_check=True)

                for kb in range(nkb):
                    kbl = kb - qt * 4
                    # bf16 operands stream at 1 row/cycle for any width, so
                    # the diagonal chunks use their exact causal width
                    c0 = max(kbl, 0) * 128
                    w = 512 - c0
                    sc = psp.tile([128, 2, 512], F32, tag="sc", bufs=2)
                    sc_s = lambda s: sc[:, s, c0:512]
                    sc_in = sc[:, :, c0:512]
                    for s in range(2):
                        po = s * 64
                        nc.tensor.matmul(
                            sc_s(s),
                            kT_sb[po:po + 64, mt, kb * 128:(kb + 1) * 128],
                            qT_t[po:po + 64, mt, c0:512],
                            start=True, stop=True,
                            tile_position=(po, 0))
                    pr = probsp.tile([128, 2, 512], BF16, tag="pr")
                    prs[kb] = pr
                    nc.scalar.activation(pr[:, :, c0:512], sc_in,
                                         AF.Exp, scale=SCALE)
                    if kbl >= 0:
                        # zero keys above the diagonal (multiply by the
                        # lower-triangular keep-mask, broadcast over heads)
                        nc.vector.tensor_mul(
                            pr[:, :, c0:c0 + 128],
                            pr[:, :, c0:c0 + 128],
                            ltri.rearrange("p (o b) -> p o b", o=1)
                                .broadcast_to([128, 2, 128]))
                    # software pipeline: av(kb-AV_LAG) is emitted after
                    # sc(kb), so the PE never stalls on an exp it just queued
                    # and the normalize of the previous pair gets runway
                    # before av reuses its psum ring
                    av_w = 0
                    if kb - AV_LAG >= 0:
                        emit_av(kb - AV_LAG)
                        kbl_p = kb - AV_LAG - qt * 4
                        av_w = 130 * (NMT - max(kbl_p, 0))
                    if kb == TP_KB and pending_tp[0] is not None:
                        # previous pair's [q,d]->[d,q] transpose (DMA): by
                        # kb2 its normalize (emitted at pair start) drained
                        pending_tp[0]()
                        pending_tp[0] = None
                    a_kb = 2 * w * ACT_EL + ACT_OVH
                    p_kb = (2 * w + av_w) * PE_CYC
                    gap_rem[0] -= max(0.0, a_kb - p_kb)
                    debt[0] += a_kb - p_kb
                    if kb <= BOOST_KB and qt >= BOOST_QT:
                        debt[0] = max(debt[0], START_BOOST)
                    if kbl >= 2:
                        debt[0] = max(debt[0], END_BOOST)
                    if qt >= QT_FLOOR:
                        debt[0] = max(debt[0], DEBT_FLOOR)
                    # reserve guard (off unless RESERVE_MARGIN set): the
                    # deps-before-rows queue order already reserves the
                    # discretionary fill for qt3
                    while debt[0] > 0 and \
                            (RESERVE_MARGIN is None
                             or inv_ns[0] > gap_rem[0] + RESERVE_MARGIN) \
                            and _advance():
                        pass
                    debt[0] = max(debt[0], -DEBT_CLAMP)
                for kb in range(nkb - AV_LAG, nkb):
                    if kb >= 0:
                        emit_av(kb)

                # normalize: per head, reciprocal of the denominator column
                # then a broadcast multiply into the [q, d] attn staging tile
                def norm():
                    asb = asbp.tile([128, NMT, 128], BF16, tag="asb",
                                    name=f"asb{qt}_{mt}")
                    dr = drp.tile([128, 2, NMT, 1], F32R, tag="dr")
                    for s in range(2):
                        with nc.allow_low_precision(reason="f32r softmax denom"):
                            nc.vector.reciprocal(dr[:, s], av[s][:, :, 64:65])
                        nc.vector.tensor_mul(
                            asb[:, :, s * 64:(s + 1) * 64],
                            av[s][:, :, 0:64],
                            dr[:, s].broadcast_to([128, NMT, 64]))

                    def tp_emit():
                        dst = attn_tiles[qt][:, mt, :]
                        if tail_mode[0]:
                            # the last pair's transpose is on the end-of-
                            # kernel critical chain: the PE+copy path beats
                            # the ~3us DMA-transpose latency there
                            tp = fill_psum(f"tp{qt}_{mt}", [128, NMT, 128],
                                           BF16)
                            for j in range(NMT):
                                nc.tensor.transpose(tp[:, j, :], asb[:, j, :],
                                                    ident[:, :])
                            nc.scalar.activation(
                                dst, tp.rearrange("p a b -> p (a b)"), AF.Copy)
                        else:
                            # xbar DMA transposes each [q,128] block to
                            # [d,128] off the PE/DVE entirely
                            nc.sync.dma_start_transpose(
                                dst.rearrange("p (a b) -> p a b", b=128),
                                asb[:, :, :])
                        tp_done[qt] += 1
                    return tp_emit
                return norm

            def r_ready(qt):
                return lambda: tp_done[qt] >= NMT

            yts = {}
            tail_mode = [False]   # after the exp stream ends, Act is free:
            tail_cnt = [0]        # alternate flushed-row copies DVE/Act
            tag_cnt = [0]         # rotate flushed-row psums across tags

            def U_row_half(qt, mt3, ntp, kc_hi=NMT):
                # half of an out-projection row on a single psum; during the
                # tail flush the attention psum tags are free, so rotating
                # units across all three tags deepens the ring pipeline and
                # removes copy-wait stalls between consecutive row units
                def g():
                    if tail_mode[0]:
                        tag_cnt[0] += 1
                        tag = ("fill", "sc", "av")[tag_cnt[0] % 3]
                        ps = psp.tile([128, 512], F32, tag=tag,
                                      bufs=(fill_bufs if tag == "fill" else 2),
                                      name=f"psy{mt3}_{ntp}")
                    else:
                        ps = fill_psum(f"psy{mt3}_{ntp}")
                    for kc in range(kc_hi):
                        nc.tensor.matmul(
                            ps,
                            attn_tiles[qt][:, kc,
                                           (mt3 % 4) * 128:(mt3 % 4 + 1) * 128],
                            wo_sb[:, kc, ntp * 512:(ntp + 1) * 512],
                            start=(kc == 0), stop=(kc == NMT - 1))
                        yield 512
                    if mt3 not in yts:
                        yts[mt3] = yp.tile([128, C], BF16, tag="y",
                                           name=f"yt{mt3}")
                    yt = yts[mt3]
                    dst = yt[:, ntp * 512:(ntp + 1) * 512]
                    tail_cnt[0] += 1
                    if tail_mode[0] and tail_cnt[0] % 2 == 0:
                        nc.scalar.activation(dst, ps, AF.Copy)
                    else:
                        nc.vector.tensor_copy(dst, ps)
                    nc.sync.dma_start(
                        out=y.ap()[mt3 * 128:(mt3 + 1) * 128,
                                   ntp * 512:(ntp + 1) * 512],
                        in_=yt[:, ntp * 512:(ntp + 1) * 512])
                return g

            # ---------------- pipelined emission ----------------
            # static filler queue in consumption order; drains enforce
            # dependencies, the in-pair pump spreads everything else into
            # Act-bound gaps.
            def _push_deps(qt):
                if qt == 0:
                    filler.append(("V0|V1|V2|V3", U_v_braid(0), None))
                    inv_ns[0] += 32 * 512 * PE_CYC
                else:
                    for i in range(qt * 4, qt * 4 + 4):
                        filler.append((f"V{i}", U_v(i), None))
                    inv_ns[0] += 4 * KC * 512 * PE_CYC
                for mt in range(NMT):
                    filler.append((f"K{mt}_{qt}", U_k(mt, qt), None))
                    filler.append((f"Q{mt}_{qt}",
                                   U_q(mt, qt, qT_tiles[qt // 2]), None))
                inv_ns[0] += 2 * NMT * KC * 512 * PE_CYC

            def _push_rows(qt):
                for m in range(4):
                    mt3 = qt * 4 + m
                    for ntp in range(2):
                        filler.append((f"R{mt3}n{ntp}",
                                       U_row_half(qt, mt3, ntp),
                                       r_ready(qt)))
                inv_ns[0] += 8 * NMT * 512 * PE_CYC

            # inventory order: ALL projection dep units first (they are
            # force-drained at pair starts anyway, so holding them back only
            # creates Act-stalling blobs), then the out-projection rows —
            # the only discretionary fill — which thereby survive to qt3
            # where the Act-over-PE gap is largest
            _push_deps(0)
            _push_deps(1)
            _push_deps(2)
            _push_deps(3)
            _push_rows(0)
            _push_rows(1)
            _push_rows(2)

            # precompute the total Act-over-PE gap the pump must fill, using
            # the same per-kb accounting as emit_attention_pair
            for _qt in range(NQT):
                for _mt in range(NMT):
                    for _kb in range(_qt * 4 + 4):
                        _w = 512 - max(_kb - _qt * 4, 0) * 128
                        _avw = 0
                        if _kb - AV_LAG >= 0:
                            _avw = 130 * (NMT - max(_kb - AV_LAG - _qt * 4, 0))
                        gap_rem[0] += max(0.0, (2 * _w * ACT_EL + ACT_OVH)
                                          - (2 * _w + _avw) * PE_CYC)

            pending_norm = None
            for qt in range(NQT):
                for mt in range(NMT):
                    if pending_norm is not None:
                        # emit the previous pair's normalize now (on DVE,
                        # ahead of the drain's filler copies); its transposes
                        # fire at kb2 of the pair emitted below
                        pending_tp[0] = pending_norm()
                        pending_norm = None
                    deps = [f"K{mt}_{qt}", f"Q{mt}_{qt}"]
                    deps += [f"V{i}" for i in range(qt * 4, qt * 4 + 4)]
                    drain(*deps)
                    pending_norm = emit_attention_pair(
                        qt, mt, qT_tiles[qt // 2][:, :, (qt % 2) * 512:
                                                  (qt % 2 + 1) * 512])

            # ---------------- tail ----------------
            # the final normalize + transposes go FIRST (the PE runtime is
            # ~10us behind emission here, so their DVE wait is already
            # satisfied when the PE reaches them), then qt3's out-projection
            # rows with their stores, and the leftover discretionary rows
            # last — by then the early stores have cleared the HWDGE queue,
            # so only the very last row's copy+DMA chain trails the PE.
            tp33 = pending_norm()
            tail_mode[0] = True
            # a few row units go first so the PE has runway while the last
            # normalize drains on DVE; then the last transposes, then the
            # rest — so rows 12..15 never stall on the tp33 copy
            for _ in range(16):
                if not _advance(force=True):
                    break
            tp33()
            while _advance(force=True):
                pass
            for m in range(4):
                for ntp in range(2):
                    for _ in U_row_half(3, 12 + m, ntp)():
                        pass
    nc.compile()
    return nc


def _shard_inputs(x, w_qkv, w_out):
    if PROJ_BF16:
        import ml_dtypes
        cast = lambda a: np.ascontiguousarray(a).astype(ml_dtypes.bfloat16)
    else:
        cast = np.ascontiguousarray
    # [C, HD] -> [partition, mt, kc, n]: element (c_in, h) with
    # c_in = kc*128 + p, h = mt*128 + n
    def _wt(a):
        return np.ascontiguousarray(
            a.reshape(KC, 128, NMT, 128).transpose(1, 2, 0, 3))

    in_maps = []
    for c in range(8):
        b, hh = c // 2, c % 2
        cols = slice(hh * HD, (hh + 1) * HD)
        in_maps.append({
            "xT": cast(x[b].T),
            "wq": _wt(cast(w_qkv[:, 0 * C:1 * C][:, cols])),
            "wk": _wt(cast(w_qkv[:, 1 * C:2 * C][:, cols])),
            "wv": cast(w_qkv[:, 2 * C:3 * C][:, cols]),
            "wo": cast(w_out[hh * HD:(hh + 1) * HD, :]),
        })
    return in_maps


def kernel(x, w_qkv, w_out):
    from concourse.bass_utils import run_bass_kernel_spmd

    x = np.asarray(x, dtype=np.float32)
    w_qkv = np.asarray(w_qkv, dtype=np.float32)
    w_out = np.asarray(w_out, dtype=np.float32)

    if "nc" not in _CACHE:
        _CACHE["nc"] = _build_nc()
    nc = _CACHE["nc"]

    in_maps = _shard_inputs(x, w_qkv, w_out)
    # the accelerator occasionally reports a transient unrecoverable state
    # after an earlier failed load; a retry clears it
    last_err = None
    for _ in range(3):
        try:
            res = run_bass_kernel_spmd(nc, in_maps, core_ids=list(range(8)))
            break
        except ModuleNotFoundError as e:
            # BASS_TRACE set in an environment without the axon NTFF hook
            last_err = e
            import os
            os.environ["BASS_NEVER_TRACE"] = "1"
        except Exception as e:
            last_err = e
            import time
            time.sleep(2.0)
    else:
        raise last_err
    outs = [np.asarray(res.results[c]["y"], dtype=np.float32) for c in range(8)]
    out = np.stack([outs[2 * b] + outs[2 * b + 1] for b in range(4)])
    return out.astype(np.float32)
